# revision 18
# baseline (speedup 1.0000x reference)
"""Linformer-style linear attention on 8 Trainium2 NeuronCores (v5 final).

Problem: B=32 heads of  softmax(Q @ (K^T E^T + e_b)/sqrt(d)) @ (F V + f_b)
with N=4096, D=128, Kp=256. Batch dim sharded 4-per-core across 8 cores
(data parallel; E_W/F_W replicated; no cross-core communication).

Measured learnings (v2 = 96.5us, v3 = 105.6us regression; see NOTES.md):
 - DMA throughput is descriptor-rate-bound below ~4KB rows (~10ns/descriptor):
   pieces must be >= 2048 bf16 cols. v3's 512-col kt pieces ran at 1/4 rate.
 - The scalar-engine HWDGE queue is much slower than sync for bulk data; only
   eb/fb (tiny) go there.
 - PE cadence = (moving_rows + ~64 LDWEIGHTS cycles) * 0.4167ns; total real PE
   work ~71us. Front half is DMA-bound (16.26MB in at ~0.42MB/us from ~8.7us).
 - Solid O-block runs are paced by the PSUM->SBUF drain engine (DVE ~930ns per
   block vs 700ns PE): route grp1 copies of the last batch through ACT (idle
   after the final exp) so the solid O(3) run is PE-bound.
 - Warmup matmuls ramp the PE p-state (1.2->2.4GHz) during the DMA fill.

v5 design (measured 92.8-93.2us, from 96.5us baseline):
 - Warmups + per-piece KP(0) waits in the fill; q0a right after ewt/kt0.
 - fwt/v4 land BEFORE kt1/q1...: VP chunks + O(0) blocks become abundant PE
   filler for the whole DMA-bound front; kt/q for later batches arrive in
   consumption order.
 - Emission alternates every ST block with ~0.7-1us of filler (VP pair, KP
   pair, O block) so ACT exp (1.3us/nt) never blocks the in-order PE queue
   through ps_st (bufs=3), and O work is pulled as early as possible.
 - All matmul operands bf16 (PSUM f32), bf16 output + host-side divide,
   batched V-projection (v4 tile), transposed scores (exp feeds O lhsT
   directly), rowsum as a ones column on V_proj, biases as rank-1 seeds.
"""

import numpy as np
import ml_dtypes

B, N, D, Kp = 32, 4096, 128, 256
NCORES = 8
BPC = B // NCORES  # 4 batches per core
SCALE = 1.0 / float(np.sqrt(D))
NT128 = N // 128   # 32
NT512 = N // 512   # 8
KC = Kp // 128     # 2
OW = 4 * (D + 1)   # 516 output cols per nt block
bf16 = ml_dtypes.bfloat16

_cache = {}


def _build_nc(bpc=BPC, debug=False):
    import concourse.bacc as bacc
    import concourse.tile as tile
    import concourse.mybir as mybir

    dt = mybir.dt
    AF = mybir.ActivationFunctionType

    nc = bacc.Bacc("TRN2", target_bir_lowering=False, debug=debug)

    qt = nc.declare_dram_parameter("qt", [bpc, D, N], dt.bfloat16, isOutput=False)
    kt = nc.declare_dram_parameter("kt", [bpc, 128, N], dt.bfloat16, isOutput=False)
    v4 = nc.declare_dram_parameter("v4", [128, NT128 * bpc * 128], dt.bfloat16, isOutput=False)
    ewt = nc.declare_dram_parameter("ewt", [128, NT128 * Kp], dt.bfloat16, isOutput=False)
    fwt = nc.declare_dram_parameter("fwt", [128, NT128 * Kp], dt.bfloat16, isOutput=False)
    eb = nc.declare_dram_parameter("eb", [1, Kp], dt.bfloat16, isOutput=False)
    fb = nc.declare_dram_parameter("fb", [1, Kp], dt.bfloat16, isOutput=False)
    # out[b, p, nt*516 + t*129 + j]: j<128 unnormalized O, j==128 rowsum,
    # for output row n = nt*512 + t*128 + p. Host divides and reorders.
    out = nc.declare_dram_parameter("out", [bpc, 128, NT512 * OW], dt.bfloat16, isOutput=True)

    with tile.TileContext(nc) as tc:
        with (
            tc.tile_pool(name="const", bufs=1) as cpool,
            tc.tile_pool(name="wq", bufs=1) as wpool,
            tc.tile_pool(name="ink", bufs=2) as kpool,
            tc.tile_pool(name="inq", bufs=3) as qpool,
            tc.tile_pool(name="kp", bufs=2) as kppool,
            tc.tile_pool(name="vext", bufs=8) as vextpool,
            tc.tile_pool(name="exp", bufs=32) as exppool,
            tc.tile_pool(name="osb", bufs=3) as opool,
            tc.tile_pool(name="ps_kp", bufs=1, space="PSUM") as ps_kp,
            tc.tile_pool(name="ps_vp", bufs=1, space="PSUM") as ps_vp,
            tc.tile_pool(name="ps_st", bufs=3, space="PSUM") as ps_st,
            tc.tile_pool(name="ps_o", bufs=2, space="PSUM") as ps_o,
        ):
            ones_sb = cpool.tile([1, 512], dt.bfloat16)
            nc.vector.memset(ones_sb[:, :], 1.0)
            # warmup operand: full 128-partition tile so warmup matmuls look
            # like real activity to the PE clock governor
            warm_sb = cpool.tile([128, 512], dt.bfloat16)
            nc.vector.memset(warm_sb[:, :], 0.0078125)
            eb_sb = cpool.tile([1, Kp], dt.bfloat16)
            nc.scalar.dma_start(eb_sb[:, :], eb[:, :])
            fb_sb = cpool.tile([1, Kp], dt.bfloat16)
            nc.scalar.dma_start(fb_sb[:, :], fb[:, :])
            ewt_sb = wpool.tile([128, NT128 * Kp], dt.bfloat16)
            fwt_sb = wpool.tile([128, NT128 * Kp], dt.bfloat16)
            v4_sb = wpool.tile([128, NT128 * bpc * 128], dt.bfloat16)

            state = {}

            def emit_warm(n):
                """PE clock-ramp / DMA-gap filler: no input deps beyond the
                warm_sb memset, output never read."""
                for _ in range(n):
                    w_ps = ps_st.tile([128, 512], dt.float32, tag="st", bufs=3)
                    nc.tensor.matmul(
                        w_ps[:, :], lhsT=warm_sb[:, 0:128], rhs=warm_sb[:, :],
                        start=True, stop=True,
                    )

            # ---------------- input DMAs (sync ring, consumption order) ----
            # All pieces have >= 4KB contiguous bytes per partition row: DMA
            # is descriptor-rate-bound (~10ns/desc) below that.
            def alloc_k(b):
                t = kpool.tile([128, N], dt.bfloat16, tag="k", name=f"k{b}")
                state[(b, "k")] = t

            def dma_k(b, j=None, pieces=1):
                t = state[(b, "k")]
                if j is None:
                    nc.sync.dma_start(t[:, :], kt[b][:, :])
                else:
                    w = N // pieces
                    nc.sync.dma_start(t[:, j * w:(j + 1) * w], kt[b][:, j * w:(j + 1) * w])

            def alloc_q(b):
                t = qpool.tile([128, N], dt.bfloat16, tag="q", name=f"q{b}")
                state[(b, "q")] = t

            def dma_q_half(b, j):
                t = state[(b, "q")]
                nc.sync.dma_start(t[:, j * 2048:(j + 1) * 2048], qt[b][:, j * 2048:(j + 1) * 2048])

            Wq = NT128 * Kp // 4          # ewt/fwt quarter: 2048 cols
            Vp8 = NT128 * bpc * 128 // 8  # v4 eighth: 2048 cols (4 c-chunks)

            def dma_ewt_piece(j):
                nc.sync.dma_start(ewt_sb[:, j * Wq:(j + 1) * Wq], ewt[:, j * Wq:(j + 1) * Wq])

            def dma_fwt_piece(j):
                nc.scalar.dma_start(fwt_sb[:, j * Wq:(j + 1) * Wq], fwt[:, j * Wq:(j + 1) * Wq])

            def dma_v4_piece(j):
                nc.sync.dma_start(v4_sb[:, j * Vp8:(j + 1) * Vp8], v4[:, j * Vp8:(j + 1) * Vp8])

            for b in range(bpc):
                alloc_k(b) if b == 0 else None
            alloc_q(0)
            # fill-phase critical path: ewt+kt0 (KP(0)), then q0a (ST(0,0..3))
            dma_ewt_piece(0)
            dma_k(0, 0, pieces=2)
            dma_ewt_piece(1)
            dma_k(0, 1, pieces=2)
            dma_ewt_piece(2)
            dma_ewt_piece(3)
            dma_q_half(0, 0)
            # VP inputs next: VP chunks + O(0) are the PE filler for the rest
            # of the DMA-bound front
            dma_fwt_piece(0)
            dma_v4_piece(0)
            dma_v4_piece(1)
            dma_fwt_piece(1)
            dma_q_half(0, 1)
            dma_v4_piece(2)
            dma_v4_piece(3)
            alloc_k(1)
            dma_k(1)
            dma_fwt_piece(2)
            dma_v4_piece(4)
            dma_v4_piece(5)
            alloc_q(1)
            dma_q_half(1, 0)
            dma_fwt_piece(3)
            dma_v4_piece(6)
            dma_v4_piece(7)
            dma_q_half(1, 1)
            alloc_k(2)
            dma_k(2)
            alloc_q(2)
            dma_q_half(2, 0)
            dma_q_half(2, 1)
            alloc_k(3)
            dma_k(3)
            alloc_q(3)
            dma_q_half(3, 0)
            dma_q_half(3, 1)

            # ---------------- compute emitters ----------------------------
            def emit_kp(b, i):
                """i in 0..7, 4 contraction chunks each. The bias rank-1 is
                emitted LAST in the accumulation group: the first chunk
                matmul must not wait on the (tiny, late-landing) eb DMA, or
                it head-of-line-blocks the whole in-order PE queue."""
                if i == 0:
                    kp_ps = ps_kp.tile([128, Kp], dt.float32, tag="kp_ps")
                    state[(b, "kp_ps")] = kp_ps
                kp_ps = state[(b, "kp_ps")]
                k_sb = state[(b, "k")]
                for c in range(4 * i, 4 * i + 4):
                    nc.tensor.matmul(
                        kp_ps[:, :],
                        lhsT=k_sb[:, c * 128:(c + 1) * 128],
                        rhs=ewt_sb[:, c * Kp:(c + 1) * Kp],
                        start=(c == 0),
                        stop=False,
                    )
                if i == 7:
                    nc.tensor.matmul(
                        kp_ps[:, :], lhsT=ones_sb[:, 0:128], rhs=eb_sb[:, :],
                        start=False, stop=True,
                    )
                    kp_sb = kppool.tile([128, Kp], dt.bfloat16, tag="kp", name=f"kp{b}")
                    nc.vector.tensor_copy(kp_sb[:, :], kp_ps[:, :])
                    state[(b, "kp")] = kp_sb

            def emit_vp_chunks(lo, hi):
                """Batched V-projection, contraction chunks [lo, hi) for both
                kc (kc-outer: consecutive matmuls stay on one PSUM bank)."""
                if lo == 0:
                    for kc in range(KC):
                        vp_ps = ps_vp.tile([128, bpc * 128], dt.float32, tag=f"vp{kc}", name=f"vp{kc}")
                        state[("vp_ps", kc)] = vp_ps
                        nc.tensor.matmul(
                            vp_ps[:, :], lhsT=fb_sb[:, kc * 128:(kc + 1) * 128],
                            rhs=ones_sb[:, :], start=True, stop=False,
                        )
                for kc in range(KC):
                    for c in range(lo, hi):
                        nc.tensor.matmul(
                            state[("vp_ps", kc)][:, :],
                            lhsT=fwt_sb[:, c * Kp + kc * 128: c * Kp + (kc + 1) * 128],
                            rhs=v4_sb[:, c * 512:(c + 1) * 512],
                            start=False,
                            stop=(c == NT128 - 1),
                        )
                if hi == NT128:
                    for b in range(bpc):
                        for kc in range(KC):
                            vext = vextpool.tile([128, D + 1], dt.bfloat16, tag=f"vext{b}_{kc}",
                                                 name=f"vext{b}_{kc}")
                            nc.vector.tensor_copy(vext[:, 0:D], state[("vp_ps", kc)][:, b * 128:(b + 1) * 128])
                            nc.vector.memset(vext[:, D:D + 1], 1.0)
                            state[(b, "vext", kc)] = vext

            def emit_st(b, nt):
                for kc in range(KC):
                    st_ps = ps_st.tile([128, 512], dt.float32, tag="st", bufs=3)
                    nc.tensor.matmul(
                        st_ps[:, :],
                        lhsT=state[(b, "kp")][:, kc * 128:(kc + 1) * 128],
                        rhs=state[(b, "q")][:, nt * 512:(nt + 1) * 512],
                        start=True, stop=True,
                    )
                    ex = exppool.tile([128, 512], dt.bfloat16, tag=f"exp{kc}", bufs=16)
                    nc.scalar.activation(ex[:, :], st_ps[:, :], AF.Exp, scale=SCALE)
                    state[(b, "exp", nt, kc)] = ex

            def emit_o(b, nt, act_grp1=False, split=False):
                """act_grp1: drain the second o_ps group via ACT instead of
                DVE — used in the final solid O run where DVE (930ns/block)
                would otherwise pace the 700ns/block PE stream and ACT is
                idle (all exps done).
                split: drain each group as two 129-col half-copies so the
                slice-level dependency releases the next block's matmul
                after only half the drain (PSUM ring is only 2 deep)."""
                out_sb = state[(b, "osb")]
                for grp in range(2):
                    o_ps = ps_o.tile([128, 2 * (D + 1)], dt.float32, tag="o_ps")
                    for tt in range(2):
                        tq = grp * 2 + tt
                        for kc in range(KC):
                            nc.tensor.matmul(
                                o_ps[:, tt * (D + 1):(tt + 1) * (D + 1)],
                                lhsT=state[(b, "exp", nt, kc)][:, tq * 128:(tq + 1) * 128],
                                rhs=state[(b, "vext", kc)][:, :],
                                start=(kc == 0),
                                stop=(kc == KC - 1),
                            )
                    base = nt * OW + grp * 2 * (D + 1)
                    use_act = act_grp1 and grp == 1
                    if split:
                        for h in range(2):
                            dst = out_sb[:, base + h * (D + 1): base + (h + 1) * (D + 1)]
                            src = o_ps[:, h * (D + 1):(h + 1) * (D + 1)]
                            if use_act:
                                nc.scalar.activation(dst, src, AF.Copy)
                            else:
                                nc.vector.tensor_copy(dst, src)
                    else:
                        dst = out_sb[:, base: base + 2 * (D + 1)]
                        if use_act:
                            nc.scalar.activation(dst, o_ps[:, :], AF.Copy)
                        else:
                            nc.vector.tensor_copy(dst, o_ps[:, :])
                for kc in range(KC):
                    del state[(b, "exp", nt, kc)]

            def alloc_osb(b):
                state[(b, "osb")] = opool.tile([128, NT512 * OW], dt.bfloat16, tag="osb", name=f"osb{b}")

            def emit_out_dma(b, pieces=1):
                t = state[(b, "osb")]
                w = NT512 * OW // pieces
                for i in range(pieces):
                    nc.sync.dma_start(out[b][:, i * w:(i + 1) * w], t[:, i * w:(i + 1) * w])

            # ---------------- emission schedule ----------------------------
            alloc_osb(0)
            alloc_osb(1)
            # Fill phase: warmups ramp the PE clock while KP(0) is DMA-paced.
            emit_warm(10)
            for i in range(8):
                emit_kp(0, i)
                emit_warm(2)
            emit_warm(2)
            # front: ST(0,0..3) gated by q0a; VP chunks arrive DMA-paced and
            # fill everything else (ACT is the ST pacer at 1.3us/nt)
            emit_st(0, 0); emit_warm(2)
            emit_st(0, 1); emit_warm(2)
            emit_st(0, 2); emit_warm(2)
            emit_st(0, 3)
            emit_vp_chunks(0, 2)
            emit_vp_chunks(2, 4)
            emit_vp_chunks(4, 6)
            emit_vp_chunks(6, 8)
            emit_st(0, 4); emit_vp_chunks(8, 10)
            emit_st(0, 5); emit_vp_chunks(10, 12)
            emit_st(0, 6); emit_vp_chunks(12, 14)
            emit_st(0, 7)
            emit_kp(1, 0); emit_kp(1, 1); emit_kp(1, 2); emit_kp(1, 3)
            emit_vp_chunks(14, 16)
            emit_kp(1, 4); emit_kp(1, 5); emit_kp(1, 6); emit_kp(1, 7)
            emit_vp_chunks(16, 18)
            emit_st(1, 0); emit_vp_chunks(18, 20)
            emit_st(1, 1); emit_vp_chunks(20, 22)
            emit_st(1, 2); emit_vp_chunks(22, 24)
            emit_st(1, 3); emit_vp_chunks(24, 26)
            emit_st(1, 4); emit_vp_chunks(26, 28)
            emit_st(1, 5); emit_vp_chunks(28, 30)
            emit_st(1, 6); emit_vp_chunks(30, 32)
            # kp groups rotate INTO the ST/O pair stream (instead of solid
            # 4us blocks) so ACT exp slack and PE filler stay balanced
            emit_st(1, 7); emit_kp(2, 0); emit_kp(2, 1)
            emit_o(0, 0); emit_kp(2, 2); emit_kp(2, 3)
            emit_o(0, 1); emit_kp(2, 4); emit_kp(2, 5)
            emit_o(0, 2); emit_kp(2, 6); emit_kp(2, 7)
            emit_st(2, 0); emit_o(0, 3)
            emit_st(2, 1); emit_o(0, 4)
            emit_st(2, 2); emit_o(0, 5)
            emit_st(2, 3); emit_o(0, 6)
            emit_st(2, 4); emit_o(0, 7)
            emit_out_dma(0, pieces=2)
            alloc_osb(2)
            emit_st(2, 5); emit_o(1, 0)
            emit_st(2, 6); emit_o(1, 1)
            emit_st(2, 7); emit_o(1, 2)
            emit_kp(3, 0); emit_kp(3, 1); emit_o(1, 3)
            emit_kp(3, 2); emit_kp(3, 3); emit_o(1, 4)
            emit_kp(3, 4); emit_kp(3, 5); emit_o(1, 5)
            emit_kp(3, 6); emit_kp(3, 7); emit_o(1, 6)
            emit_st(3, 0); emit_o(1, 7)
            emit_out_dma(1, pieces=2)
            alloc_osb(3)

            def drain3(a):
                t3 = state[(3, "osb")]
                nc.sync.dma_start(out[3][:, a * OW:(a + 2) * OW], t3[:, a * OW:(a + 2) * OW])

            # Unwind the O backlog DURING the ST(3,*) cycles (1 ST + 2 O =
            # 1.9us PE per cycle, drains fully overlapped) instead of a solid
            # drain-paced O(3) run after the last ST (was ~1.0us/block vs the
            # 0.7 PE floor). exp(3,nt) is produced 2+ cycles ahead of its use.
            emit_st(3, 1); emit_o(2, 0)
            emit_st(3, 2); emit_o(2, 1); emit_o(3, 0)
            emit_st(3, 3); emit_o(2, 2); emit_o(3, 1)
            emit_st(3, 4); emit_o(2, 3); emit_o(3, 2); drain3(0)
            emit_st(3, 5); emit_o(2, 4); emit_o(3, 3)
            emit_st(3, 6); emit_o(2, 5); emit_o(3, 4); drain3(2)
            emit_st(3, 7); emit_o(2, 6); emit_o(3, 5)
            emit_o(2, 7, act_grp1=True); emit_o(3, 6); drain3(4)
            emit_out_dma(2, pieces=2)
            emit_o(3, 7, act_grp1=True)
            drain3(6)

    nc.compile()
    return nc


def _prep(Q, K, V, E_W, E_b, F_W, F_b):
    """Host-side: cast to bf16 and pre-tile so every DMA is contiguous."""
    QT = np.ascontiguousarray(Q.astype(bf16).transpose(0, 2, 1))       # [B, D, N]
    Kt = np.ascontiguousarray(
        K.astype(bf16).reshape(B, NT128, 128, D).transpose(0, 2, 1, 3)
    ).reshape(B, 128, N)
    # v4 per core: [p, (c, b_local, j)] from V[core slice]
    V4 = np.ascontiguousarray(
        V.astype(bf16).reshape(NCORES, BPC, NT128, 128, D).transpose(0, 3, 2, 1, 4)
    ).reshape(NCORES, 128, NT128 * BPC * D)
    EWT = np.ascontiguousarray(
        E_W.T.astype(bf16).reshape(NT128, 128, Kp).transpose(1, 0, 2)
    ).reshape(128, NT128 * Kp)
    FWT = np.ascontiguousarray(
        F_W.T.astype(bf16).reshape(NT128, 128, Kp).transpose(1, 0, 2)
    ).reshape(128, NT128 * Kp)
    ebh = E_b.astype(bf16).reshape(1, Kp)
    fbh = F_b.astype(bf16).reshape(1, Kp)
    return QT, Kt, V4, EWT, FWT, ebh, fbh


def _postprocess(raw):
    """raw [nb, 128, NT512*516] bf16 -> normalized O [nb, N, D] f32."""
    nb = raw.shape[0]
    r = raw.astype(np.float32).reshape(nb, 128, NT512, 4, D + 1)
    r = r.transpose(0, 2, 3, 1, 4)            # [nb, nt, t, p, D+1]
    r = r.reshape(nb, N, D + 1)
    return (r[:, :, :D] / r[:, :, D:D + 1]).astype(np.float32)


def kernel(Q, K, V, E_W, E_b, F_W, F_b):
    QT, Kt, V4, EWT, FWT, ebh, fbh = _prep(Q, K, V, E_W, E_b, F_W, F_b)

    if "nc" not in _cache:
        _cache["nc"] = _build_nc()
    nc = _cache["nc"]

    in_maps = []
    for i in range(NCORES):
        sl = slice(i * BPC, (i + 1) * BPC)
        in_maps.append({
            "qt": QT[sl], "kt": Kt[sl], "v4": V4[i],
            "ewt": EWT, "fwt": FWT, "eb": ebh, "fb": fbh,
        })

    from concourse.bass_utils import run_bass_kernel_spmd

    res = run_bass_kernel_spmd(nc, in_maps, list(range(NCORES)))
    kernel.last_result = res
    kernel.last_exec_time_ns = res.exec_time_ns

    raw = np.stack([np.asarray(res.results[i]["out"]) for i in range(NCORES)], axis=0)
    raw = raw.reshape(B, 128, NT512 * OW)
    return np.ascontiguousarray(_postprocess(raw))


# revision 19
# speedup vs baseline: 1.0438x; 1.0438x over previous
"""Linformer-style linear attention on 8 Trainium2 NeuronCores (v5 final).

Problem: B=32 heads of  softmax(Q @ (K^T E^T + e_b)/sqrt(d)) @ (F V + f_b)
with N=4096, D=128, Kp=256. Batch dim sharded 4-per-core across 8 cores
(data parallel; E_W/F_W replicated; no cross-core communication).

Measured learnings (v2 = 96.5us, v3 = 105.6us regression; see NOTES.md):
 - DMA throughput is descriptor-rate-bound below ~4KB rows (~10ns/descriptor):
   pieces must be >= 2048 bf16 cols. v3's 512-col kt pieces ran at 1/4 rate.
 - The scalar-engine HWDGE queue is much slower than sync for bulk data; only
   eb/fb (tiny) go there.
 - PE cadence = (moving_rows + ~64 LDWEIGHTS cycles) * 0.4167ns; total real PE
   work ~71us. Front half is DMA-bound (16.26MB in at ~0.42MB/us from ~8.7us).
 - Solid O-block runs are paced by the PSUM->SBUF drain engine (DVE ~930ns per
   block vs 700ns PE): route grp1 copies of the last batch through ACT (idle
   after the final exp) so the solid O(3) run is PE-bound.
 - Warmup matmuls ramp the PE p-state (1.2->2.4GHz) during the DMA fill.

v5 design (measured 92.8-93.2us, from 96.5us baseline):
 - Warmups + per-piece KP(0) waits in the fill; q0a right after ewt/kt0.
 - fwt/v4 land BEFORE kt1/q1...: VP chunks + O(0) blocks become abundant PE
   filler for the whole DMA-bound front; kt/q for later batches arrive in
   consumption order.
 - Emission alternates every ST block with ~0.7-1us of filler (VP pair, KP
   pair, O block) so ACT exp (1.3us/nt) never blocks the in-order PE queue
   through ps_st (bufs=3), and O work is pulled as early as possible.
 - All matmul operands bf16 (PSUM f32), bf16 output + host-side divide,
   batched V-projection (v4 tile), transposed scores (exp feeds O lhsT
   directly), rowsum as a ones column on V_proj, biases as rank-1 seeds.
"""

import numpy as np
import ml_dtypes

B, N, D, Kp = 32, 4096, 128, 256
NCORES = 8
BPC = B // NCORES  # 4 batches per core
SCALE = 1.0 / float(np.sqrt(D))
NT128 = N // 128   # 32
NT512 = N // 512   # 8
KC = Kp // 128     # 2
OW = 4 * (D + 1)   # 516 output cols per nt block
bf16 = ml_dtypes.bfloat16

_cache = {}


def _build_nc(bpc=BPC, debug=False):
    import concourse.bacc as bacc
    import concourse.tile as tile
    import concourse.mybir as mybir

    dt = mybir.dt
    AF = mybir.ActivationFunctionType

    nc = bacc.Bacc("TRN2", target_bir_lowering=False, debug=debug)

    qt = nc.declare_dram_parameter("qt", [bpc, D, N], dt.bfloat16, isOutput=False)
    kt = nc.declare_dram_parameter("kt", [bpc, 128, N], dt.bfloat16, isOutput=False)
    v4 = nc.declare_dram_parameter("v4", [128, NT128 * bpc * 128], dt.bfloat16, isOutput=False)
    ewt = nc.declare_dram_parameter("ewt", [128, NT128 * Kp], dt.bfloat16, isOutput=False)
    fwt = nc.declare_dram_parameter("fwt", [128, NT128 * Kp], dt.bfloat16, isOutput=False)
    eb = nc.declare_dram_parameter("eb", [1, Kp], dt.bfloat16, isOutput=False)
    fb = nc.declare_dram_parameter("fb", [1, Kp], dt.bfloat16, isOutput=False)
    # out[b, p, nt*516 + t*129 + j]: j<128 unnormalized O, j==128 rowsum,
    # for output row n = nt*512 + t*128 + p. Host divides and reorders.
    out = nc.declare_dram_parameter("out", [bpc, 128, NT512 * OW], dt.bfloat16, isOutput=True)

    with tile.TileContext(nc) as tc:
        with (
            tc.tile_pool(name="const", bufs=1) as cpool,
            tc.tile_pool(name="wq", bufs=1) as wpool,
            tc.tile_pool(name="ink", bufs=2) as kpool,
            tc.tile_pool(name="inq", bufs=3) as qpool,
            tc.tile_pool(name="kp", bufs=2) as kppool,
            tc.tile_pool(name="vext", bufs=8) as vextpool,
            tc.tile_pool(name="exp", bufs=32) as exppool,
            tc.tile_pool(name="osb", bufs=3) as opool,
            tc.tile_pool(name="ps_kp", bufs=1, space="PSUM") as ps_kp,
            tc.tile_pool(name="ps_vp", bufs=1, space="PSUM") as ps_vp,
            tc.tile_pool(name="ps_st", bufs=3, space="PSUM") as ps_st,
            tc.tile_pool(name="ps_o", bufs=2, space="PSUM") as ps_o,
        ):
            ones_sb = cpool.tile([1, 512], dt.bfloat16)
            nc.vector.memset(ones_sb[:, :], 1.0)
            # warmup operand: full 128-partition tile so warmup matmuls look
            # like real activity to the PE clock governor
            warm_sb = cpool.tile([128, 512], dt.bfloat16)
            nc.vector.memset(warm_sb[:, :], 0.0078125)
            eb_sb = cpool.tile([1, Kp], dt.bfloat16)
            nc.scalar.dma_start(eb_sb[:, :], eb[:, :])
            fb_sb = cpool.tile([1, Kp], dt.bfloat16)
            nc.scalar.dma_start(fb_sb[:, :], fb[:, :])
            ewt_sb = wpool.tile([128, NT128 * Kp], dt.bfloat16)
            fwt_sb = wpool.tile([128, NT128 * Kp], dt.bfloat16)
            v4_sb = wpool.tile([128, NT128 * bpc * 128], dt.bfloat16)

            state = {}

            def emit_warm(n):
                """PE clock-ramp / DMA-gap filler: no input deps beyond the
                warm_sb memset, output never read."""
                for _ in range(n):
                    w_ps = ps_st.tile([128, 512], dt.float32, tag="st", bufs=3)
                    nc.tensor.matmul(
                        w_ps[:, :], lhsT=warm_sb[:, 0:128], rhs=warm_sb[:, :],
                        start=True, stop=True,
                    )

            # ---------------- input DMAs (sync ring, consumption order) ----
            # All pieces have >= 4KB contiguous bytes per partition row: DMA
            # is descriptor-rate-bound (~10ns/desc) below that.
            def alloc_k(b):
                t = kpool.tile([128, N], dt.bfloat16, tag="k", name=f"k{b}")
                state[(b, "k")] = t

            def dma_k(b, j=None, pieces=1):
                t = state[(b, "k")]
                if j is None:
                    nc.sync.dma_start(t[:, :], kt[b][:, :])
                else:
                    w = N // pieces
                    nc.sync.dma_start(t[:, j * w:(j + 1) * w], kt[b][:, j * w:(j + 1) * w])

            def alloc_q(b):
                t = qpool.tile([128, N], dt.bfloat16, tag="q", name=f"q{b}")
                state[(b, "q")] = t

            def dma_q_half(b, j):
                t = state[(b, "q")]
                nc.sync.dma_start(t[:, j * 2048:(j + 1) * 2048], qt[b][:, j * 2048:(j + 1) * 2048])

            Wq = NT128 * Kp // 4          # ewt/fwt quarter: 2048 cols
            Vp8 = NT128 * bpc * 128 // 8  # v4 eighth: 2048 cols (4 c-chunks)

            def dma_ewt_piece(j):
                nc.sync.dma_start(ewt_sb[:, j * Wq:(j + 1) * Wq], ewt[:, j * Wq:(j + 1) * Wq])

            def dma_fwt_piece(j):
                nc.sync.dma_start(fwt_sb[:, j * Wq:(j + 1) * Wq], fwt[:, j * Wq:(j + 1) * Wq])

            def dma_v4_piece(j):
                nc.sync.dma_start(v4_sb[:, j * Vp8:(j + 1) * Vp8], v4[:, j * Vp8:(j + 1) * Vp8])

            for b in range(bpc):
                alloc_k(b) if b == 0 else None
            alloc_q(0)
            # fill-phase critical path: ewt+kt0 (KP(0)), then q0a (ST(0,0..3))
            dma_ewt_piece(0)
            dma_k(0, 0, pieces=2)
            dma_ewt_piece(1)
            dma_k(0, 1, pieces=2)
            dma_ewt_piece(2)
            dma_ewt_piece(3)
            dma_q_half(0, 0)
            # VP inputs next: VP chunks + O(0) are the PE filler for the rest
            # of the DMA-bound front
            dma_fwt_piece(0)
            dma_v4_piece(0)
            dma_v4_piece(1)
            dma_fwt_piece(1)
            dma_q_half(0, 1)
            dma_v4_piece(2)
            dma_v4_piece(3)
            alloc_k(1)
            dma_k(1)
            dma_fwt_piece(2)
            dma_v4_piece(4)
            dma_v4_piece(5)
            alloc_q(1)
            dma_q_half(1, 0)
            dma_fwt_piece(3)
            dma_v4_piece(6)
            dma_v4_piece(7)
            dma_q_half(1, 1)
            alloc_k(2)
            dma_k(2)
            alloc_q(2)
            dma_q_half(2, 0)
            dma_q_half(2, 1)
            alloc_k(3)
            dma_k(3)
            alloc_q(3)
            dma_q_half(3, 0)
            dma_q_half(3, 1)

            # ---------------- compute emitters ----------------------------
            def emit_kp(b, i):
                """i in 0..7, 4 contraction chunks each. The bias rank-1 is
                emitted LAST in the accumulation group: the first chunk
                matmul must not wait on the (tiny, late-landing) eb DMA, or
                it head-of-line-blocks the whole in-order PE queue."""
                if i == 0:
                    kp_ps = ps_kp.tile([128, Kp], dt.float32, tag="kp_ps")
                    state[(b, "kp_ps")] = kp_ps
                kp_ps = state[(b, "kp_ps")]
                k_sb = state[(b, "k")]
                for c in range(4 * i, 4 * i + 4):
                    nc.tensor.matmul(
                        kp_ps[:, :],
                        lhsT=k_sb[:, c * 128:(c + 1) * 128],
                        rhs=ewt_sb[:, c * Kp:(c + 1) * Kp],
                        start=(c == 0),
                        stop=False,
                    )
                if i == 7:
                    nc.tensor.matmul(
                        kp_ps[:, :], lhsT=ones_sb[:, 0:128], rhs=eb_sb[:, :],
                        start=False, stop=True,
                    )
                    kp_sb = kppool.tile([128, Kp], dt.bfloat16, tag="kp", name=f"kp{b}")
                    nc.vector.tensor_copy(kp_sb[:, :], kp_ps[:, :])
                    state[(b, "kp")] = kp_sb

            def emit_vp_chunks(lo, hi):
                """Batched V-projection, contraction chunks [lo, hi) for both
                kc (kc-outer: consecutive matmuls stay on one PSUM bank)."""
                if lo == 0:
                    for kc in range(KC):
                        vp_ps = ps_vp.tile([128, bpc * 128], dt.float32, tag=f"vp{kc}", name=f"vp{kc}")
                        state[("vp_ps", kc)] = vp_ps
                        nc.tensor.matmul(
                            vp_ps[:, :], lhsT=fb_sb[:, kc * 128:(kc + 1) * 128],
                            rhs=ones_sb[:, :], start=True, stop=False,
                        )
                for kc in range(KC):
                    for c in range(lo, hi):
                        nc.tensor.matmul(
                            state[("vp_ps", kc)][:, :],
                            lhsT=fwt_sb[:, c * Kp + kc * 128: c * Kp + (kc + 1) * 128],
                            rhs=v4_sb[:, c * 512:(c + 1) * 512],
                            start=False,
                            stop=(c == NT128 - 1),
                        )
                if hi == NT128:
                    for b in range(bpc):
                        for kc in range(KC):
                            vext = vextpool.tile([128, D + 1], dt.bfloat16, tag=f"vext{b}_{kc}",
                                                 name=f"vext{b}_{kc}")
                            nc.vector.tensor_copy(vext[:, 0:D], state[("vp_ps", kc)][:, b * 128:(b + 1) * 128])
                            nc.vector.memset(vext[:, D:D + 1], 1.0)
                            state[(b, "vext", kc)] = vext

            def emit_st(b, nt):
                for kc in range(KC):
                    st_ps = ps_st.tile([128, 512], dt.float32, tag="st", bufs=3)
                    nc.tensor.matmul(
                        st_ps[:, :],
                        lhsT=state[(b, "kp")][:, kc * 128:(kc + 1) * 128],
                        rhs=state[(b, "q")][:, nt * 512:(nt + 1) * 512],
                        start=True, stop=True,
                    )
                    ex = exppool.tile([128, 512], dt.bfloat16, tag=f"exp{kc}", bufs=16)
                    nc.scalar.activation(ex[:, :], st_ps[:, :], AF.Exp, scale=SCALE)
                    state[(b, "exp", nt, kc)] = ex

            def emit_o(b, nt, act_grp1=False, split=False):
                """act_grp1: drain the second o_ps group via ACT instead of
                DVE — used in the final solid O run where DVE (930ns/block)
                would otherwise pace the 700ns/block PE stream and ACT is
                idle (all exps done).
                split: drain each group as two 129-col half-copies so the
                slice-level dependency releases the next block's matmul
                after only half the drain (PSUM ring is only 2 deep)."""
                out_sb = state[(b, "osb")]
                for grp in range(2):
                    o_ps = ps_o.tile([128, 2 * (D + 1)], dt.float32, tag="o_ps")
                    for tt in range(2):
                        tq = grp * 2 + tt
                        for kc in range(KC):
                            nc.tensor.matmul(
                                o_ps[:, tt * (D + 1):(tt + 1) * (D + 1)],
                                lhsT=state[(b, "exp", nt, kc)][:, tq * 128:(tq + 1) * 128],
                                rhs=state[(b, "vext", kc)][:, :],
                                start=(kc == 0),
                                stop=(kc == KC - 1),
                            )
                    base = nt * OW + grp * 2 * (D + 1)
                    use_act = act_grp1 and grp == 1
                    if split:
                        for h in range(2):
                            dst = out_sb[:, base + h * (D + 1): base + (h + 1) * (D + 1)]
                            src = o_ps[:, h * (D + 1):(h + 1) * (D + 1)]
                            if use_act:
                                nc.scalar.activation(dst, src, AF.Copy)
                            else:
                                nc.vector.tensor_copy(dst, src)
                    else:
                        dst = out_sb[:, base: base + 2 * (D + 1)]
                        if use_act:
                            nc.scalar.activation(dst, o_ps[:, :], AF.Copy)
                        else:
                            nc.vector.tensor_copy(dst, o_ps[:, :])
                for kc in range(KC):
                    del state[(b, "exp", nt, kc)]

            def alloc_osb(b):
                state[(b, "osb")] = opool.tile([128, NT512 * OW], dt.bfloat16, tag="osb", name=f"osb{b}")

            def emit_out_dma(b, pieces=1):
                t = state[(b, "osb")]
                w = NT512 * OW // pieces
                for i in range(pieces):
                    nc.sync.dma_start(out[b][:, i * w:(i + 1) * w], t[:, i * w:(i + 1) * w])

            # ---------------- emission schedule ----------------------------
            alloc_osb(0)
            alloc_osb(1)
            # Fill phase: warmups ramp the PE clock while KP(0) is DMA-paced.
            emit_warm(10)
            for i in range(8):
                emit_kp(0, i)
                emit_warm(2)
            emit_warm(2)
            # front: ST(0,0..3) gated by q0a; VP chunks arrive DMA-paced and
            # fill everything else (ACT is the ST pacer at 1.3us/nt)
            emit_st(0, 0); emit_warm(2)
            emit_st(0, 1); emit_warm(2)
            emit_st(0, 2); emit_warm(2)
            emit_st(0, 3)
            emit_vp_chunks(0, 2)
            emit_vp_chunks(2, 4)
            emit_vp_chunks(4, 6)
            emit_vp_chunks(6, 8)
            emit_st(0, 4); emit_vp_chunks(8, 10)
            emit_st(0, 5); emit_vp_chunks(10, 12)
            emit_st(0, 6); emit_vp_chunks(12, 14)
            emit_st(0, 7)
            emit_kp(1, 0); emit_kp(1, 1); emit_kp(1, 2); emit_kp(1, 3)
            emit_vp_chunks(14, 16)
            emit_kp(1, 4); emit_kp(1, 5); emit_kp(1, 6); emit_kp(1, 7)
            emit_vp_chunks(16, 18)
            emit_st(1, 0); emit_vp_chunks(18, 20)
            emit_st(1, 1); emit_vp_chunks(20, 22)
            emit_st(1, 2); emit_vp_chunks(22, 24)
            emit_st(1, 3); emit_vp_chunks(24, 26)
            emit_st(1, 4); emit_vp_chunks(26, 28)
            emit_st(1, 5); emit_vp_chunks(28, 30)
            emit_st(1, 6); emit_vp_chunks(30, 32)
            # kp groups rotate INTO the ST/O pair stream (instead of solid
            # 4us blocks) so ACT exp slack and PE filler stay balanced
            emit_st(1, 7); emit_kp(2, 0); emit_kp(2, 1)
            emit_o(0, 0); emit_kp(2, 2); emit_kp(2, 3)
            emit_o(0, 1); emit_kp(2, 4); emit_kp(2, 5)
            emit_o(0, 2); emit_kp(2, 6); emit_kp(2, 7)
            emit_st(2, 0); emit_o(0, 3)
            emit_st(2, 1); emit_o(0, 4)
            emit_st(2, 2); emit_o(0, 5)
            emit_st(2, 3); emit_o(0, 6)
            emit_st(2, 4); emit_o(0, 7)
            emit_out_dma(0, pieces=2)
            alloc_osb(2)
            emit_st(2, 5); emit_o(1, 0)
            emit_st(2, 6); emit_o(1, 1)
            emit_st(2, 7); emit_o(1, 2)
            emit_kp(3, 0); emit_kp(3, 1); emit_o(1, 3)
            emit_kp(3, 2); emit_kp(3, 3); emit_o(1, 4)
            emit_kp(3, 4); emit_kp(3, 5); emit_o(1, 5)
            emit_kp(3, 6); emit_kp(3, 7); emit_o(1, 6)
            emit_st(3, 0); emit_o(1, 7)
            emit_out_dma(1, pieces=2)
            alloc_osb(3)

            def drain3(a):
                t3 = state[(3, "osb")]
                nc.sync.dma_start(out[3][:, a * OW:(a + 2) * OW], t3[:, a * OW:(a + 2) * OW])

            # Unwind the O backlog DURING the ST(3,*) cycles (1 ST + 2 O =
            # 1.9us PE per cycle, drains fully overlapped) instead of a solid
            # drain-paced O(3) run after the last ST (was ~1.0us/block vs the
            # 0.7 PE floor). exp(3,nt) is produced 2+ cycles ahead of its use.
            emit_st(3, 1); emit_o(2, 0)
            emit_st(3, 2); emit_o(2, 1); emit_o(3, 0)
            emit_st(3, 3); emit_o(2, 2); emit_o(3, 1)
            emit_st(3, 4); emit_o(2, 3); emit_o(3, 2); drain3(0)
            emit_st(3, 5); emit_o(2, 4); emit_o(3, 3)
            emit_st(3, 6); emit_o(2, 5); emit_o(3, 4); drain3(2)
            emit_st(3, 7); emit_o(2, 6); emit_o(3, 5)
            emit_o(2, 7, act_grp1=True); emit_o(3, 6); drain3(4)
            emit_out_dma(2, pieces=2)
            emit_o(3, 7, act_grp1=True)
            drain3(6)

    nc.compile()
    return nc


def _prep(Q, K, V, E_W, E_b, F_W, F_b):
    """Host-side: cast to bf16 and pre-tile so every DMA is contiguous."""
    QT = np.ascontiguousarray(Q.astype(bf16).transpose(0, 2, 1))       # [B, D, N]
    Kt = np.ascontiguousarray(
        K.astype(bf16).reshape(B, NT128, 128, D).transpose(0, 2, 1, 3)
    ).reshape(B, 128, N)
    # v4 per core: [p, (c, b_local, j)] from V[core slice]
    V4 = np.ascontiguousarray(
        V.astype(bf16).reshape(NCORES, BPC, NT128, 128, D).transpose(0, 3, 2, 1, 4)
    ).reshape(NCORES, 128, NT128 * BPC * D)
    EWT = np.ascontiguousarray(
        E_W.T.astype(bf16).reshape(NT128, 128, Kp).transpose(1, 0, 2)
    ).reshape(128, NT128 * Kp)
    FWT = np.ascontiguousarray(
        F_W.T.astype(bf16).reshape(NT128, 128, Kp).transpose(1, 0, 2)
    ).reshape(128, NT128 * Kp)
    ebh = E_b.astype(bf16).reshape(1, Kp)
    fbh = F_b.astype(bf16).reshape(1, Kp)
    return QT, Kt, V4, EWT, FWT, ebh, fbh


def _postprocess(raw):
    """raw [nb, 128, NT512*516] bf16 -> normalized O [nb, N, D] f32."""
    nb = raw.shape[0]
    r = raw.astype(np.float32).reshape(nb, 128, NT512, 4, D + 1)
    r = r.transpose(0, 2, 3, 1, 4)            # [nb, nt, t, p, D+1]
    r = r.reshape(nb, N, D + 1)
    return (r[:, :, :D] / r[:, :, D:D + 1]).astype(np.float32)


def kernel(Q, K, V, E_W, E_b, F_W, F_b):
    QT, Kt, V4, EWT, FWT, ebh, fbh = _prep(Q, K, V, E_W, E_b, F_W, F_b)

    if "nc" not in _cache:
        _cache["nc"] = _build_nc()
    nc = _cache["nc"]

    in_maps = []
    for i in range(NCORES):
        sl = slice(i * BPC, (i + 1) * BPC)
        in_maps.append({
            "qt": QT[sl], "kt": Kt[sl], "v4": V4[i],
            "ewt": EWT, "fwt": FWT, "eb": ebh, "fb": fbh,
        })

    from concourse.bass_utils import run_bass_kernel_spmd

    res = run_bass_kernel_spmd(nc, in_maps, list(range(NCORES)))
    kernel.last_result = res
    kernel.last_exec_time_ns = res.exec_time_ns

    raw = np.stack([np.asarray(res.results[i]["out"]) for i in range(NCORES)], axis=0)
    raw = raw.reshape(B, 128, NT512 * OW)
    return np.ascontiguousarray(_postprocess(raw))


# revision 20
# speedup vs baseline: 1.0444x; 1.0006x over previous
"""Linformer-style linear attention on 8 Trainium2 NeuronCores (final).

Problem: B=32 heads of  softmax(Q @ (K^T E^T + e_b)/sqrt(d)) @ (F V + f_b)
with N=4096, D=128, Kp=256. Batch dim sharded 4-per-core across 8 cores
(data parallel; E_W/F_W replicated; no cross-core communication).

Measured learnings (v2 = 96.5us, v3 = 105.6us regression; see NOTES.md):
 - DMA throughput is descriptor-rate-bound below ~4KB rows (~10ns/descriptor):
   pieces must be >= 2048 bf16 cols. v3's 512-col kt pieces ran at 1/4 rate.
 - The scalar-engine HWDGE queue is much slower than sync for bulk data; only
   eb/fb (tiny) go there.
 - PE cadence = (moving_rows + ~64 LDWEIGHTS cycles) * 0.4167ns; total real PE
   work ~71us. Front half is DMA-bound (16.26MB in at ~0.42MB/us from ~8.7us).
 - Solid O-block runs are paced by the PSUM->SBUF drain engine (DVE ~930ns per
   block vs 700ns PE): route grp1 copies of the last batch through ACT (idle
   after the final exp) so the solid O(3) run is PE-bound.
 - Warmup matmuls ramp the PE p-state (1.2->2.4GHz) during the DMA fill.

Final design (measured 90.9-92.5us clean runs, from 96.5us baseline):
 - Warmups + per-piece KP(0) waits in the fill; q0a right after ewt/kt0.
 - fwt/v4 land BEFORE kt1/q1...: VP chunks + O(0) blocks become abundant PE
   filler for the whole DMA-bound front; kt/q for later batches arrive in
   consumption order.
 - Emission alternates every ST block with ~0.7-1us of filler (VP pair, KP
   pair, O block) so ACT exp (1.3us/nt) never blocks the in-order PE queue
   through ps_st (bufs=3), and O work is pulled as early as possible.
 - Tail: O(3,nt) blocks ride inside the ST(3,nt+2) cycles (1 ST + 2 O =
   ~1.9us PE per cycle) so PSUM drains fully overlap; only ~2 O blocks run
   solo after the last ST. opool bufs=3 lets osb3 skip the out1 wait.
 - DMA fabric is ~420GB/s SHARED across HWDGE queues: all bulk data stays on
   the sync queue, in consumption order, pieces >= 4KB per partition row.
 - All matmul operands bf16 (PSUM f32), bf16 output + host-side divide,
   batched V-projection (v4 tile), transposed scores (exp feeds O lhsT
   directly), rowsum as a ones column on V_proj, biases as rank-1 seeds.
"""

import numpy as np
import ml_dtypes

B, N, D, Kp = 32, 4096, 128, 256
NCORES = 8
BPC = B // NCORES  # 4 batches per core
SCALE = 1.0 / float(np.sqrt(D))
NT128 = N // 128   # 32
NT512 = N // 512   # 8
KC = Kp // 128     # 2
OW = 4 * (D + 1)   # 516 output cols per nt block
bf16 = ml_dtypes.bfloat16

_cache = {}


def _build_nc(bpc=BPC, debug=False):
    import concourse.bacc as bacc
    import concourse.tile as tile
    import concourse.mybir as mybir

    dt = mybir.dt
    AF = mybir.ActivationFunctionType

    nc = bacc.Bacc("TRN2", target_bir_lowering=False, debug=debug)

    qt = nc.declare_dram_parameter("qt", [bpc, D, N], dt.bfloat16, isOutput=False)
    kt = nc.declare_dram_parameter("kt", [bpc, 128, N], dt.bfloat16, isOutput=False)
    v4 = nc.declare_dram_parameter("v4", [128, NT128 * bpc * 128], dt.bfloat16, isOutput=False)
    ewt = nc.declare_dram_parameter("ewt", [128, NT128 * Kp], dt.bfloat16, isOutput=False)
    fwt = nc.declare_dram_parameter("fwt", [128, NT128 * Kp], dt.bfloat16, isOutput=False)
    eb = nc.declare_dram_parameter("eb", [1, Kp], dt.bfloat16, isOutput=False)
    fb = nc.declare_dram_parameter("fb", [1, Kp], dt.bfloat16, isOutput=False)
    # out[b, p, nt*516 + t*129 + j]: j<128 unnormalized O, j==128 rowsum,
    # for output row n = nt*512 + t*128 + p. Host divides and reorders.
    out = nc.declare_dram_parameter("out", [bpc, 128, NT512 * OW], dt.bfloat16, isOutput=True)

    with tile.TileContext(nc) as tc:
        with (
            tc.tile_pool(name="const", bufs=1) as cpool,
            tc.tile_pool(name="wq", bufs=1) as wpool,
            tc.tile_pool(name="ink", bufs=2) as kpool,
            tc.tile_pool(name="inq", bufs=3) as qpool,
            tc.tile_pool(name="kp", bufs=2) as kppool,
            tc.tile_pool(name="vext", bufs=8) as vextpool,
            tc.tile_pool(name="exp", bufs=32) as exppool,
            tc.tile_pool(name="osb", bufs=3) as opool,
            tc.tile_pool(name="ps_kp", bufs=1, space="PSUM") as ps_kp,
            tc.tile_pool(name="ps_vp", bufs=1, space="PSUM") as ps_vp,
            tc.tile_pool(name="ps_st", bufs=3, space="PSUM") as ps_st,
            tc.tile_pool(name="ps_o", bufs=2, space="PSUM") as ps_o,
        ):
            ones_sb = cpool.tile([1, 512], dt.bfloat16)
            nc.vector.memset(ones_sb[:, :], 1.0)
            # warmup operand: full 128-partition tile so warmup matmuls look
            # like real activity to the PE clock governor
            warm_sb = cpool.tile([128, 512], dt.bfloat16)
            nc.vector.memset(warm_sb[:, :], 0.0078125)
            eb_sb = cpool.tile([1, Kp], dt.bfloat16)
            nc.scalar.dma_start(eb_sb[:, :], eb[:, :])
            fb_sb = cpool.tile([1, Kp], dt.bfloat16)
            nc.scalar.dma_start(fb_sb[:, :], fb[:, :])
            ewt_sb = wpool.tile([128, NT128 * Kp], dt.bfloat16)
            fwt_sb = wpool.tile([128, NT128 * Kp], dt.bfloat16)
            v4_sb = wpool.tile([128, NT128 * bpc * 128], dt.bfloat16)

            state = {}

            def emit_warm(n):
                """PE clock-ramp / DMA-gap filler: no input deps beyond the
                warm_sb memset, output never read."""
                for _ in range(n):
                    w_ps = ps_st.tile([128, 512], dt.float32, tag="st", bufs=3)
                    nc.tensor.matmul(
                        w_ps[:, :], lhsT=warm_sb[:, 0:128], rhs=warm_sb[:, :],
                        start=True, stop=True,
                    )

            # ---------------- input DMAs (sync ring, consumption order) ----
            # All pieces have >= 4KB contiguous bytes per partition row: DMA
            # is descriptor-rate-bound (~10ns/desc) below that.
            def alloc_k(b):
                t = kpool.tile([128, N], dt.bfloat16, tag="k", name=f"k{b}")
                state[(b, "k")] = t

            def dma_k(b, j=None, pieces=1):
                t = state[(b, "k")]
                if j is None:
                    nc.sync.dma_start(t[:, :], kt[b][:, :])
                else:
                    w = N // pieces
                    nc.sync.dma_start(t[:, j * w:(j + 1) * w], kt[b][:, j * w:(j + 1) * w])

            def alloc_q(b):
                t = qpool.tile([128, N], dt.bfloat16, tag="q", name=f"q{b}")
                state[(b, "q")] = t

            def dma_q_half(b, j):
                t = state[(b, "q")]
                nc.sync.dma_start(t[:, j * 2048:(j + 1) * 2048], qt[b][:, j * 2048:(j + 1) * 2048])

            Wq = NT128 * Kp // 4          # ewt/fwt quarter: 2048 cols
            Vp8 = NT128 * bpc * 128 // 8  # v4 eighth: 2048 cols (4 c-chunks)

            def dma_ewt_piece(j):
                nc.sync.dma_start(ewt_sb[:, j * Wq:(j + 1) * Wq], ewt[:, j * Wq:(j + 1) * Wq])

            def dma_fwt_piece(j):
                nc.sync.dma_start(fwt_sb[:, j * Wq:(j + 1) * Wq], fwt[:, j * Wq:(j + 1) * Wq])

            def dma_v4_piece(j):
                nc.sync.dma_start(v4_sb[:, j * Vp8:(j + 1) * Vp8], v4[:, j * Vp8:(j + 1) * Vp8])

            for b in range(bpc):
                alloc_k(b) if b == 0 else None
            alloc_q(0)
            # fill-phase critical path: ewt+kt0 (KP(0)), then q0a (ST(0,0..3))
            dma_ewt_piece(0)
            dma_k(0, 0, pieces=2)
            dma_ewt_piece(1)
            dma_k(0, 1, pieces=2)
            dma_ewt_piece(2)
            dma_ewt_piece(3)
            dma_q_half(0, 0)
            # VP inputs next: VP chunks + O(0) are the PE filler for the rest
            # of the DMA-bound front
            dma_fwt_piece(0)
            dma_v4_piece(0)
            dma_v4_piece(1)
            dma_fwt_piece(1)
            dma_q_half(0, 1)
            dma_v4_piece(2)
            dma_v4_piece(3)
            alloc_k(1)
            dma_k(1)
            dma_fwt_piece(2)
            dma_v4_piece(4)
            dma_v4_piece(5)
            alloc_q(1)
            dma_q_half(1, 0)
            dma_fwt_piece(3)
            dma_v4_piece(6)
            dma_v4_piece(7)
            dma_q_half(1, 1)
            alloc_k(2)
            dma_k(2)
            alloc_q(2)
            dma_q_half(2, 0)
            dma_q_half(2, 1)
            alloc_k(3)
            dma_k(3)
            alloc_q(3)
            dma_q_half(3, 0)
            dma_q_half(3, 1)

            # ---------------- compute emitters ----------------------------
            def emit_kp(b, i):
                """i in 0..7, 4 contraction chunks each. The bias rank-1 is
                emitted LAST in the accumulation group: the first chunk
                matmul must not wait on the (tiny, late-landing) eb DMA, or
                it head-of-line-blocks the whole in-order PE queue."""
                if i == 0:
                    kp_ps = ps_kp.tile([128, Kp], dt.float32, tag="kp_ps")
                    state[(b, "kp_ps")] = kp_ps
                kp_ps = state[(b, "kp_ps")]
                k_sb = state[(b, "k")]
                for c in range(4 * i, 4 * i + 4):
                    nc.tensor.matmul(
                        kp_ps[:, :],
                        lhsT=k_sb[:, c * 128:(c + 1) * 128],
                        rhs=ewt_sb[:, c * Kp:(c + 1) * Kp],
                        start=(c == 0),
                        stop=False,
                    )
                if i == 7:
                    nc.tensor.matmul(
                        kp_ps[:, :], lhsT=ones_sb[:, 0:128], rhs=eb_sb[:, :],
                        start=False, stop=True,
                    )
                    kp_sb = kppool.tile([128, Kp], dt.bfloat16, tag="kp", name=f"kp{b}")
                    nc.vector.tensor_copy(kp_sb[:, :], kp_ps[:, :])
                    state[(b, "kp")] = kp_sb

            def emit_vp_chunks(lo, hi):
                """Batched V-projection, contraction chunks [lo, hi) for both
                kc (kc-outer: consecutive matmuls stay on one PSUM bank)."""
                if lo == 0:
                    for kc in range(KC):
                        vp_ps = ps_vp.tile([128, bpc * 128], dt.float32, tag=f"vp{kc}", name=f"vp{kc}")
                        state[("vp_ps", kc)] = vp_ps
                        nc.tensor.matmul(
                            vp_ps[:, :], lhsT=fb_sb[:, kc * 128:(kc + 1) * 128],
                            rhs=ones_sb[:, :], start=True, stop=False,
                        )
                for kc in range(KC):
                    for c in range(lo, hi):
                        nc.tensor.matmul(
                            state[("vp_ps", kc)][:, :],
                            lhsT=fwt_sb[:, c * Kp + kc * 128: c * Kp + (kc + 1) * 128],
                            rhs=v4_sb[:, c * 512:(c + 1) * 512],
                            start=False,
                            stop=(c == NT128 - 1),
                        )
                if hi == NT128:
                    for b in range(bpc):
                        for kc in range(KC):
                            vext = vextpool.tile([128, D + 1], dt.bfloat16, tag=f"vext{b}_{kc}",
                                                 name=f"vext{b}_{kc}")
                            nc.vector.tensor_copy(vext[:, 0:D], state[("vp_ps", kc)][:, b * 128:(b + 1) * 128])
                            nc.vector.memset(vext[:, D:D + 1], 1.0)
                            state[(b, "vext", kc)] = vext

            def emit_st(b, nt):
                for kc in range(KC):
                    st_ps = ps_st.tile([128, 512], dt.float32, tag="st", bufs=3)
                    nc.tensor.matmul(
                        st_ps[:, :],
                        lhsT=state[(b, "kp")][:, kc * 128:(kc + 1) * 128],
                        rhs=state[(b, "q")][:, nt * 512:(nt + 1) * 512],
                        start=True, stop=True,
                    )
                    ex = exppool.tile([128, 512], dt.bfloat16, tag=f"exp{kc}", bufs=16)
                    nc.scalar.activation(ex[:, :], st_ps[:, :], AF.Exp, scale=SCALE)
                    state[(b, "exp", nt, kc)] = ex

            def emit_o(b, nt, act_grp1=False, split=False):
                """act_grp1: drain the second o_ps group via ACT instead of
                DVE — used in the final solid O run where DVE (930ns/block)
                would otherwise pace the 700ns/block PE stream and ACT is
                idle (all exps done).
                split: drain each group as two 129-col half-copies so the
                slice-level dependency releases the next block's matmul
                after only half the drain (PSUM ring is only 2 deep)."""
                out_sb = state[(b, "osb")]
                for grp in range(2):
                    o_ps = ps_o.tile([128, 2 * (D + 1)], dt.float32, tag="o_ps")
                    for tt in range(2):
                        tq = grp * 2 + tt
                        for kc in range(KC):
                            nc.tensor.matmul(
                                o_ps[:, tt * (D + 1):(tt + 1) * (D + 1)],
                                lhsT=state[(b, "exp", nt, kc)][:, tq * 128:(tq + 1) * 128],
                                rhs=state[(b, "vext", kc)][:, :],
                                start=(kc == 0),
                                stop=(kc == KC - 1),
                            )
                    base = nt * OW + grp * 2 * (D + 1)
                    use_act = act_grp1 and grp == 1
                    if split:
                        for h in range(2):
                            dst = out_sb[:, base + h * (D + 1): base + (h + 1) * (D + 1)]
                            src = o_ps[:, h * (D + 1):(h + 1) * (D + 1)]
                            if use_act:
                                nc.scalar.activation(dst, src, AF.Copy)
                            else:
                                nc.vector.tensor_copy(dst, src)
                    else:
                        dst = out_sb[:, base: base + 2 * (D + 1)]
                        if use_act:
                            nc.scalar.activation(dst, o_ps[:, :], AF.Copy)
                        else:
                            nc.vector.tensor_copy(dst, o_ps[:, :])
                for kc in range(KC):
                    del state[(b, "exp", nt, kc)]

            def alloc_osb(b):
                state[(b, "osb")] = opool.tile([128, NT512 * OW], dt.bfloat16, tag="osb", name=f"osb{b}")

            def emit_out_dma(b, pieces=1):
                t = state[(b, "osb")]
                w = NT512 * OW // pieces
                for i in range(pieces):
                    nc.sync.dma_start(out[b][:, i * w:(i + 1) * w], t[:, i * w:(i + 1) * w])

            # ---------------- emission schedule ----------------------------
            alloc_osb(0)
            alloc_osb(1)
            # Fill phase: warmups ramp the PE clock while KP(0) is DMA-paced.
            emit_warm(10)
            for i in range(8):
                emit_kp(0, i)
                emit_warm(2)
            emit_warm(2)
            # front: ST(0,0..3) gated by q0a; VP chunks arrive DMA-paced and
            # fill everything else (ACT is the ST pacer at 1.3us/nt)
            emit_st(0, 0); emit_warm(2)
            emit_st(0, 1); emit_warm(2)
            emit_st(0, 2); emit_warm(2)
            emit_st(0, 3)
            emit_vp_chunks(0, 2)
            emit_vp_chunks(2, 4)
            emit_vp_chunks(4, 6)
            emit_vp_chunks(6, 8)
            emit_st(0, 4); emit_vp_chunks(8, 10)
            emit_st(0, 5); emit_vp_chunks(10, 12)
            emit_st(0, 6); emit_vp_chunks(12, 14)
            emit_st(0, 7)
            emit_kp(1, 0); emit_kp(1, 1); emit_kp(1, 2); emit_kp(1, 3)
            emit_vp_chunks(14, 16)
            emit_kp(1, 4); emit_kp(1, 5); emit_kp(1, 6); emit_kp(1, 7)
            emit_vp_chunks(16, 18)
            emit_st(1, 0); emit_vp_chunks(18, 20)
            emit_st(1, 1); emit_vp_chunks(20, 22)
            emit_st(1, 2); emit_vp_chunks(22, 24)
            emit_st(1, 3); emit_vp_chunks(24, 26)
            emit_st(1, 4); emit_vp_chunks(26, 28)
            emit_st(1, 5); emit_vp_chunks(28, 30)
            emit_st(1, 6); emit_vp_chunks(30, 32)
            # kp groups rotate INTO the ST/O pair stream (instead of solid
            # 4us blocks) so ACT exp slack and PE filler stay balanced
            emit_st(1, 7); emit_kp(2, 0); emit_kp(2, 1)
            emit_o(0, 0); emit_kp(2, 2); emit_kp(2, 3)
            emit_o(0, 1); emit_kp(2, 4); emit_kp(2, 5)
            emit_o(0, 2); emit_kp(2, 6); emit_kp(2, 7)
            emit_st(2, 0); emit_o(0, 3)
            emit_st(2, 1); emit_o(0, 4)
            emit_st(2, 2); emit_o(0, 5)
            emit_st(2, 3); emit_o(0, 6)
            emit_st(2, 4); emit_o(0, 7)
            emit_out_dma(0, pieces=2)
            alloc_osb(2)
            emit_st(2, 5); emit_o(1, 0)
            emit_st(2, 6); emit_o(1, 1)
            emit_st(2, 7); emit_o(1, 2)
            emit_kp(3, 0); emit_kp(3, 1); emit_o(1, 3)
            emit_kp(3, 2); emit_kp(3, 3); emit_o(1, 4)
            emit_kp(3, 4); emit_kp(3, 5); emit_o(1, 5)
            emit_kp(3, 6); emit_kp(3, 7); emit_o(1, 6)
            emit_st(3, 0); emit_o(1, 7)
            emit_out_dma(1, pieces=2)
            alloc_osb(3)

            def drain3(a):
                t3 = state[(3, "osb")]
                nc.sync.dma_start(out[3][:, a * OW:(a + 2) * OW], t3[:, a * OW:(a + 2) * OW])

            # Unwind the O backlog DURING the ST(3,*) cycles (1 ST + 2 O =
            # 1.9us PE per cycle, drains fully overlapped) instead of a solid
            # drain-paced O(3) run after the last ST (was ~1.0us/block vs the
            # 0.7 PE floor). exp(3,nt) is produced 2+ cycles ahead of its use.
            emit_st(3, 1); emit_o(2, 0)
            emit_st(3, 2); emit_o(2, 1); emit_o(3, 0)
            emit_st(3, 3); emit_o(2, 2); emit_o(3, 1)
            emit_st(3, 4); emit_o(2, 3); emit_o(3, 2); drain3(0)
            emit_st(3, 5); emit_o(2, 4); emit_o(3, 3)
            emit_st(3, 6); emit_o(2, 5); emit_o(3, 4); drain3(2)
            emit_st(3, 7); emit_o(2, 6); emit_o(3, 5)
            emit_o(2, 7, act_grp1=True); emit_o(3, 6); drain3(4)
            emit_out_dma(2, pieces=2)
            emit_o(3, 7, act_grp1=True)
            drain3(6)

    nc.compile()
    return nc


def _prep(Q, K, V, E_W, E_b, F_W, F_b):
    """Host-side: cast to bf16 and pre-tile so every DMA is contiguous."""
    QT = np.ascontiguousarray(Q.astype(bf16).transpose(0, 2, 1))       # [B, D, N]
    Kt = np.ascontiguousarray(
        K.astype(bf16).reshape(B, NT128, 128, D).transpose(0, 2, 1, 3)
    ).reshape(B, 128, N)
    # v4 per core: [p, (c, b_local, j)] from V[core slice]
    V4 = np.ascontiguousarray(
        V.astype(bf16).reshape(NCORES, BPC, NT128, 128, D).transpose(0, 3, 2, 1, 4)
    ).reshape(NCORES, 128, NT128 * BPC * D)
    EWT = np.ascontiguousarray(
        E_W.T.astype(bf16).reshape(NT128, 128, Kp).transpose(1, 0, 2)
    ).reshape(128, NT128 * Kp)
    FWT = np.ascontiguousarray(
        F_W.T.astype(bf16).reshape(NT128, 128, Kp).transpose(1, 0, 2)
    ).reshape(128, NT128 * Kp)
    ebh = E_b.astype(bf16).reshape(1, Kp)
    fbh = F_b.astype(bf16).reshape(1, Kp)
    return QT, Kt, V4, EWT, FWT, ebh, fbh


def _postprocess(raw):
    """raw [nb, 128, NT512*516] bf16 -> normalized O [nb, N, D] f32."""
    nb = raw.shape[0]
    r = raw.astype(np.float32).reshape(nb, 128, NT512, 4, D + 1)
    r = r.transpose(0, 2, 3, 1, 4)            # [nb, nt, t, p, D+1]
    r = r.reshape(nb, N, D + 1)
    return (r[:, :, :D] / r[:, :, D:D + 1]).astype(np.float32)


def kernel(Q, K, V, E_W, E_b, F_W, F_b):
    QT, Kt, V4, EWT, FWT, ebh, fbh = _prep(Q, K, V, E_W, E_b, F_W, F_b)

    if "nc" not in _cache:
        _cache["nc"] = _build_nc()
    nc = _cache["nc"]

    in_maps = []
    for i in range(NCORES):
        sl = slice(i * BPC, (i + 1) * BPC)
        in_maps.append({
            "qt": QT[sl], "kt": Kt[sl], "v4": V4[i],
            "ewt": EWT, "fwt": FWT, "eb": ebh, "fb": fbh,
        })

    from concourse.bass_utils import run_bass_kernel_spmd

    res = run_bass_kernel_spmd(nc, in_maps, list(range(NCORES)))
    kernel.last_result = res
    kernel.last_exec_time_ns = res.exec_time_ns

    raw = np.stack([np.asarray(res.results[i]["out"]) for i in range(NCORES)], axis=0)
    raw = raw.reshape(B, 128, NT512 * OW)
    return np.ascontiguousarray(_postprocess(raw))
